# revision 40
# baseline (speedup 1.0000x reference)
# Bass/Tile TRN2 kernel for nn_BiasedCrossDecoderLayer (dense cross-attention
# transformer decoder layer), SPMD over 8 NeuronCores.
#
# Sharding: core c -> batch b = c//4, head-group hg = c%4 (4 of 16 heads =
# 256 of 1024 qkv feature dims).  Attention is head-parallel; the
# out-projection produces partial sums which are ReduceScattered (along the
# query axis) within each 4-core batch group; the FFN then runs
# sequence-parallel on each core's 256-query slice with the full 4096 hidden.
# Each core ends up owning queries [512H + 128r, +128) for H in {0,1}.
#
# v3 highlights:
#  - All matmul operands bf16 (same PE rate, half DMA/SBUF, 2x DVE); LN stat
#    rows fp32r.  softmax is exp(logits) * exp(mask) with exp(mask) from host.
#  - Attention in two query-halves, each half's out-proj feeds a bf16
#    ReduceScatter immediately so RS(half0) hides under attention(half1).
#  - The V projection is braided into attention(half0, head0)'s instruction
#    stream and small filler matmuls pad the other head loops: the PE never
#    idles, which keeps the HAM clock gate at 2.4 GHz (idle-y streams measure
#    ~1.1 GHz).
#  - FFN runs operand-swapped: stationary activations, weights as the moving
#    rhs (N=512), so LDWEIGHTS is off the critical path.  FFN1 emits gelu via
#    the ACT scale port (per-token 1/std lives on partitions in token-major);
#    g is PE-transposed back to ff-major for FFN2; the final bias is a K=1
#    matmul appended to the FFN2 accumulation.  Output leaves token-major.
#  - reciprocal_approx_fast (18 bits) instead of the 3.3us DVE reciprocal.
#
# LayerNorm folding (host):  q = LN(x;g,b) @ Wq.T + pq == LN0(x) @ Wq'.T + b'
# with Wq' = wq*g (attention 1/sqrt(hd) folded in), b' = wq@b + pq.  In
# feature-major layout qT = rB * (Wq' @ xT_raw + ADJ) where
# ADJ[o,t] = -rowsum(Wq')[o]*m[t] + b'[o]*std[t] is a K=2 matmul appended to
# the PSUM accumulation.  V is produced token-major with the analogous fold.
# The softmax denominator is a ones-column appended to V (M=65 matmul).

import os
import sys

import numpy as np

sys.path.insert(0, "/opt/trn_rl_repo")

import concourse.bass as bass  # noqa: E402
import concourse.mybir as mybir  # noqa: E402
import concourse.tile as tile  # noqa: E402
from concourse import bacc  # noqa: E402

F32 = mybir.dt.float32
F32R = mybir.dt.float32r
BF16 = mybir.dt.bfloat16
AF = mybir.ActivationFunctionType
ALU = mybir.AluOpType

B, Q, S, D, H = 2, 1024, 2048, 1024, 16
HD = D // H       # 64
FF = 4 * D
EPS = 1e-5
NCORES = 8
NH = 4            # heads per core
FC = NH * HD      # 256 qkv feature dims per core
QS = 256          # owned queries per core (two 128-slices, one per Q-half)
P = 128
KX = D // P       # 8 k-tiles over the model dim
QH = Q // 2       # 512-query attention half

REPLICA_GROUPS = [[0, 1, 2, 3], [4, 5, 6, 7]]

LAST_RESULT = None  # BassKernelResults of the most recent run (for test.py)


def _r(ap):
    """View an fp32 AP as float32r for full-rate PE matmuls."""
    return ap.bitcast(F32R)


def build_nc():
    nc = bacc.Bacc(
        "TRN2",
        target_bir_lowering=False,
        debug=False,
        num_devices=NCORES,
        name="biased_cross_decoder",
    )

    # ---- DRAM I/O (per-core shards; same program on all cores) ----
    d = {}
    d["xT"] = nc.dram_tensor("xT", [D, Q], BF16, kind="ExternalInput").ap()
    d["zT"] = nc.dram_tensor("zT", [D, S], BF16, kind="ExternalInput").ap()
    d["emaskT"] = nc.dram_tensor(
        "emaskT", [NH, 2, S // P, P, QH], BF16, kind="ExternalInput").ap()
    d["wqT"] = nc.dram_tensor("wqT", [P, KX, FC], BF16, kind="ExternalInput").ap()
    d["wkT"] = nc.dram_tensor("wkT", [P, KX, FC], BF16, kind="ExternalInput").ap()
    d["wvT"] = nc.dram_tensor("wvT", [P, KX, FC], BF16, kind="ExternalInput").ap()
    d["adjq"] = nc.dram_tensor("adjq", [2, FC], F32R, kind="ExternalInput").ap()
    d["adjk"] = nc.dram_tensor("adjk", [2, FC], F32R, kind="ExternalInput").ap()
    d["adjv"] = nc.dram_tensor("adjv", [2, FC], F32R, kind="ExternalInput").ap()
    d["owh"] = nc.dram_tensor("owh", [HD, NH, D], BF16, kind="ExternalInput").ap()
    d["outb"] = nc.dram_tensor("outb", [D], F32, kind="ExternalInput").ap()
    d["xq"] = nc.dram_tensor("xq", [D, QS], F32, kind="ExternalInput").ap()
    d["w1T"] = nc.dram_tensor("w1T", [D, FF], BF16, kind="ExternalInput").ap()
    d["adjf"] = nc.dram_tensor("adjf", [2, FF], F32R, kind="ExternalInput").ap()
    d["w2T"] = nc.dram_tensor("w2T", [FF, D], BF16, kind="ExternalInput").ap()
    d["b2r"] = nc.dram_tensor("b2r", [1, D], F32R, kind="ExternalInput").ap()
    d["ident"] = nc.dram_tensor("ident", [P, P], BF16, kind="ExternalInput").ap()
    d["out"] = nc.dram_tensor("out", [QS, D], F32, kind="ExternalOutput").ap()

    with tile.TileContext(nc) as tc:
        build_tile_program(tc, nc, d)
    nc.compile()   # bacc passes: wait splitting, ldweights fusion, reg alloc
    return nc


class _Pool:
    """Keeps the tile_pool context manager alive; allows explicit close."""

    def __init__(self, cm):
        self._cm = cm
        self.pool = cm.__enter__()

    def tile(self, *a, **kw):
        kw.setdefault("name", kw.get("tag") or "t")
        return self.pool.tile(*a, **kw)

    def close(self):
        self._cm.__exit__(None, None, None)


def build_tile_program(tc, nc, d):
    # ---------------- persistent constants ----------------
    const = _Pool(tc.tile_pool(name="const", bufs=1))
    dram = _Pool(tc.tile_pool(name="dram", bufs=1, space="DRAM"))

    ones_sb = const.tile([P, P], BF16, tag="ones_sb")
    nc.vector.memset(ones_sb, 1.0)
    ones_col = ones_sb[:, 0:1]                  # lhsT for column sums
    ones32 = const.tile([P, P], F32, tag="ones32")   # f32r lhsT for broadcasts
    nc.vector.memset(ones32, 1.0)
    eps_t = const.tile([1, 1], F32, tag="eps")
    nc.vector.memset(eps_t, EPS)
    outb_col = const.tile([P, KX], F32, tag="outb_col")
    nc.sync.dma_start(outb_col, d["outb"].rearrange("(o p) -> p o", p=P))
    rz_col = const.tile([P, S // P], F32, tag="rz_col")     # rstd_z token-striped

    rs_scr = dram.tile([1, S], F32, tag="rs_scr")           # row restripe bounce
    rs_in = [dram.tile([4, D, QH // 4], BF16, name=f"rs_in{i}", tag=f"rs_in{i}")
             for i in range(2)]
    rs_out = [dram.tile([D, QH // 4], BF16, name=f"rs_out{i}", tag=f"rs_out{i}")
              for i in range(2)]

    # ---------------- long-lived right-side pools ----------------
    pool_w1 = _Pool(tc.tile_pool(name="w1", bufs=1, side="right"))
    w1_sb = pool_w1.tile([P, KX, FF], BF16, tag="w1_sb")    # loaded in attn phase
    pool_qkv = _Pool(tc.tile_pool(name="qkv", bufs=1, side="right"))
    qT = pool_qkv.tile([P, FC // P, Q], BF16, tag="qT")   # includes 1/8 scale
    kT = pool_qkv.tile([P, FC // P, S], BF16, tag="kT")
    v_sb = pool_qkv.tile([P, S // P, NH, HD + 1], BF16, tag="v_sb")

    # ---------------- front-phase scratch pools (left stack) ----------------
    # stack order (bottom->top): wq, adj, pz | rows, sq, rb, px
    # the right of the bar closes at front-phase end; the left survives the
    # attention braid (v-proj needs zT/adjz/wv) and closes after attention.
    pool_wq = _Pool(tc.tile_pool(name="wq", bufs=1))
    pool_adj = _Pool(tc.tile_pool(name="adj", bufs=1))
    pool_z = _Pool(tc.tile_pool(name="pz", bufs=1))
    pool_rows = _Pool(tc.tile_pool(name="rows", bufs=1))
    pool_sq = _Pool(tc.tile_pool(name="sq", bufs=3))
    pool_rb = _Pool(tc.tile_pool(name="rb", bufs=6))      # 1/std row-broadcasts
    pool_x = _Pool(tc.tile_pool(name="px", bufs=1))

    adjx = pool_adj.tile([2, Q], F32R, tag="adjx")        # [mx ; stdx]
    adjz = pool_adj.tile([2, S], F32R, tag="adjz")        # [mz ; stdz]

    zT = pool_z.tile([P, KX, S], BF16, tag="zT")
    xT = pool_x.tile([P, KX, Q], BF16, tag="xT")
    for ch in range(2):
        for k in range(KX):
            nc.sync.dma_start(xT[:, k, ch * 512:(ch + 1) * 512],
                              d["xT"][k * P:(k + 1) * P, ch * 512:(ch + 1) * 512])
    for ch in range(4):
        for k in range(KX):
            nc.sync.dma_start(zT[:, k, ch * 512:(ch + 1) * 512],
                              d["zT"][k * P:(k + 1) * P, ch * 512:(ch + 1) * 512])

    def ln_chunk_stats(aT, sl, adj, ps_stats, scr=None):
        """LN stats for one 512-token chunk: adj[:, sl] = [mean ; std]; returns
        the 1/std broadcast tile [P, 512] (SBUF, f32)."""
        w = sl.stop - sl.start
        ps_sum = ps_stats.tile([1, w], F32, name="ps_sum", tag="ps_sum")
        ps_ssq = ps_stats.tile([1, w], F32, name="ps_ssq", tag="ps_ssq")
        for k in range(KX):
            nc.tensor.matmul(ps_sum, ones_col, aT[:, k, sl],
                             start=(k == 0), stop=(k == KX - 1))
        for k in range(KX):
            sq = pool_sq.tile([P, w], BF16, name="sq", tag="sq")
            nc.vector.tensor_mul(sq, aT[:, k, sl], aT[:, k, sl])
            nc.tensor.matmul(ps_ssq, ones_col, sq,
                             start=(k == 0), stop=(k == KX - 1))
        m2 = pool_rows.tile([1, w], F32, name="m2", tag="m2")
        e2 = pool_rows.tile([1, w], F32, name="e2", tag="e2")
        inv = pool_rows.tile([1, w], F32R, name="inv", tag="inv")
        rr = pool_rows.tile([1, w], F32, name="rr", tag="rr")
        rrr = pool_rows.tile([1, w], F32R, name="rrr", tag="rrr")
        nc.vector.tensor_scalar_mul(adj[0:1, sl], ps_sum, 1.0 / D)  # mean
        nc.vector.tensor_mul(m2, adj[0:1, sl].bitcast(F32), adj[0:1, sl].bitcast(F32))
        nc.vector.scalar_tensor_tensor(out=e2, in0=ps_ssq, scalar=1.0 / D,
                                       in1=m2, op0=ALU.mult, op1=ALU.subtract)
        nc.scalar.activation(inv, e2, AF.Sqrt, bias=eps_t[0:1])     # std
        nc.vector.reciprocal_approx_fast(rr, inv.bitcast(F32))
        nc.scalar.copy(rrr, rr)              # f32r-rounded copy for the PE
        nc.sync.dma_start(adj[1:2, sl], inv)  # cross-partition row move
        bcp = ps_stats.tile([P, w], F32, name="bcp", tag="bcp")
        nc.tensor.matmul(bcp, _r(ones32[0:1, :]), rrr)
        rB = pool_rb.tile([P, w], F32, name="rB", tag="rB")
        nc.scalar.copy(rB, bcp)
        if scr is not None:
            nc.sync.dma_start(scr[0:1, sl], rr)
        return rB

    # =============== front phase: x stats+qproj, z stats, k proj ===============
    ps_qk = _Pool(tc.tile_pool(name="ps_qk", bufs=1, space="PSUM"))
    ps_st = _Pool(tc.tile_pool(name="ps_st", bufs=1, space="PSUM"))

    wq_sb = pool_wq.tile([P, KX, FC], BF16, tag="wq_sb")
    nc.sync.dma_start(wq_sb, d["wqT"])
    wk_sb = pool_wq.tile([P, KX, FC], BF16, tag="wk_sb")
    nc.sync.dma_start(wk_sb, d["wkT"])
    wv_sb = pool_wq.tile([P, KX, FC], BF16, tag="wv_sb")
    nc.sync.dma_start(wv_sb, d["wvT"])
    adjq_w = pool_wq.tile([2, FC], F32R, tag="adjq_w")
    nc.sync.dma_start(adjq_w, d["adjq"])
    adjk_w = pool_wq.tile([2, FC], F32R, tag="adjk_w")
    nc.sync.dma_start(adjk_w, d["adjk"])
    adjv_w = pool_wq.tile([2, FC], F32R, tag="adjv_w")
    nc.sync.dma_start(adjv_w, d["adjv"])

    # x: stats + q projection, 512-token chunks
    for ch in range(2):
        sl = slice(ch * 512, (ch + 1) * 512)
        rxB = ln_chunk_stats(xT, sl, adjx, ps_st)
        pss = []
        for m in range(FC // P):
            ps = ps_qk.tile([P, 512], F32, name=f"ps_q{m}", tag=f"ps_q{m}")
            for k in range(KX):
                nc.tensor.matmul(ps, wq_sb[:, k, m * P:(m + 1) * P],
                                 xT[:, k, sl], start=(k == 0), stop=False)
            pss.append(ps)
        for m in range(FC // P):
            nc.tensor.matmul(pss[m], _r(adjq_w[:, m * P:(m + 1) * P]),
                             _r(adjx[:, sl]), start=False, stop=True)
            nc.vector.tensor_mul(qT[:, m, sl], pss[m], rxB)

    # z: stats for all chunks, then k projection
    rzBs = []
    for ch in range(4):
        sl = slice(ch * 512, (ch + 1) * 512)
        rzBs.append(ln_chunk_stats(zT, sl, adjz, ps_st, scr=rs_scr))
    # restripe 1/std_z to token-major columns for the v normalize
    nc.sync.dma_start(rz_col, rs_scr.rearrange("a (i p) -> (a p) i", p=P))

    for ch in range(4):
        sl = slice(ch * 512, (ch + 1) * 512)
        for m in range(FC // P):
            ps = ps_qk.tile([P, 512], F32, name=f"ps_q{m}", tag=f"ps_q{m}")
            for k in range(KX):
                nc.tensor.matmul(ps, wk_sb[:, k, m * P:(m + 1) * P],
                                 zT[:, k, sl], start=(k == 0), stop=False)
            nc.tensor.matmul(ps, _r(adjk_w[:, m * P:(m + 1) * P]),
                             _r(adjz[:, sl]), start=False, stop=True)
            nc.vector.tensor_mul(kT[:, m, sl], ps, rzBs[ch])

    # front-phase scratch no longer needed; free before attention pools open
    pool_x.close()
    pool_rb.close()
    pool_sq.close()
    pool_rows.close()
    ps_st.close()
    ps_qk.close()


    # softmax-denominator ones column
    nc.vector.memset(v_sb[:, :, :, HD:HD + 1], 1.0)

    def v_proj(t):
        """One token-tile of the V projection (braided into attention h0)."""
        ps = pool_ps_v.tile([P, FC], F32, name="ps_v_t", tag="ps_v_t")
        for k in range(KX):
            nc.tensor.matmul(ps, zT[:, k, t * P:(t + 1) * P],
                             wv_sb[:, k, :], start=(k == 0), stop=False)
        nc.tensor.matmul(ps, _r(adjz[:, t * P:(t + 1) * P]), _r(adjv_w),
                         start=False, stop=True)
        nc.vector.tensor_scalar_mul(
            v_sb[:, t, :, 0:HD],
            ps.rearrange("p (h e) -> p h e", h=NH),
            rz_col[:, t:t + 1])

    # ============ attention + out-projection + RS, per query-half ============
    pool_mask = _Pool(tc.tile_pool(name="mask", bufs=4))
    pool_eg = _Pool(tc.tile_pool(name="eg", bufs=3))
    pool_pr = _Pool(tc.tile_pool(name="probs", bufs=4))
    pool_a64 = _Pool(tc.tile_pool(name="a64", bufs=2))
    pool_nbc = _Pool(tc.tile_pool(name="nbc", bufs=2))
    pool_rrow = _Pool(tc.tile_pool(name="rrow", bufs=2))
    pool_ow = _Pool(tc.tile_pool(name="ow", bufs=1))
    pool_osb = _Pool(tc.tile_pool(name="osb", bufs=3))
    ps_lg = _Pool(tc.tile_pool(name="ps_lg", bufs=2, space="PSUM"))
    ps_att = _Pool(tc.tile_pool(name="ps_att", bufs=2, space="PSUM"))
    ps_nbc = _Pool(tc.tile_pool(name="ps_nbc", bufs=1, space="PSUM"))
    ps_dum = _Pool(tc.tile_pool(name="ps_dum", bufs=1, space="PSUM"))
    pool_ps_v = _Pool(tc.tile_pool(name="ps_v", bufs=1, space="PSUM"))

    ow_sb = pool_ow.tile([HD, NH, D], BF16, tag="ow_sb")
    nc.sync.dma_start(ow_sb, d["owh"])
    v_proj(0)
    v_proj(1)
    dum = ps_dum.tile([HD + 1, QH], F32, tag="dum")

    for half in range(2):
        qsl = slice(half * QH, (half + 1) * QH)
        att64 = []
        for h in range(NH):
            ht, ho = h // 2, HD * (h % 2)
            att_ps = ps_att.tile([HD + 1, QH], F32, name="att_ps", tag="att_ps")
            prs = [None] * (S // P)
            braid_v = (half == 0 and h == 0)
            for st in range(S // P):
                if braid_v and st + 2 < S // P:
                    v_proj(st + 2)
                mk = pool_mask.tile([P, QH], BF16, name="mk", tag="mk")
                nc.sync.dma_start(mk, d["emaskT"][h, half, st])
                lg = ps_lg.tile([P, QH], F32, name="lg", tag="lg")
                nc.tensor.matmul(
                    lg, kT[ho:ho + HD, ht, st * P:(st + 1) * P],
                    qT[ho:ho + HD, ht, qsl])
                eg = pool_eg.tile([P, QH], BF16, name="eg", tag="eg")
                nc.scalar.activation(eg, lg, AF.Exp)
                pr = pool_pr.tile([P, QH], BF16, name="pr", tag="pr")
                nc.vector.tensor_mul(pr, eg, mk)
                prs[st] = pr
                # PV accumulation two steps behind: the exp+mul chain is
                # ~1.1us, more than one 2.4 GHz iteration period, so a
                # one-step pipeline still stalls the PE every iteration
                if st > 1:
                    nc.tensor.matmul(att_ps, v_sb[:, st - 2, h, :],
                                     prs[st - 2], start=(st == 2), stop=False)
                    prs[st - 2] = None
                if not braid_v:
                    # filler matmul: keeps the PE 100%-busy (HAM re-throttles
                    # on ANY idle in its 3.4us window; a gappy stream locks
                    # the clock at 1.2 GHz and the whole phase runs 2x slow)
                    nc.tensor.matmul(dum, v_sb[:, st, h, :], mk)
            for st in (S // P - 2, S // P - 1):
                nc.tensor.matmul(att_ps, v_sb[:, st, h, :], prs[st],
                                 start=False, stop=(st == S // P - 1))
            # normalize: att[0:64] * broadcast(1 / att[64])
            rr = pool_rrow.tile([HD + 1, QH], F32R, name="rr", tag="rr")
            with nc.allow_low_precision(reason="fp32r rounding of 1/sum"):
                nc.vector.reciprocal(rr[HD:HD + 1, :], att_ps[HD:HD + 1, :])
            bc = ps_nbc.tile([HD, QH], F32, name="bc2", tag="bc2")
            nc.tensor.matmul(bc, _r(ones32[HD:HD + 1, 0:HD]),
                             rr[HD:HD + 1, :])
            nbc = pool_nbc.tile([HD, QH], F32, name="nbc_t", tag="nbc_t")
            nc.scalar.copy(nbc, bc)
            a64 = pool_a64.tile([HD, QH], BF16, name=f"a64_{h}", tag=f"a64_{h}")
            nc.vector.tensor_mul(a64, att_ps[0:HD, :], nbc)
            att64.append(a64)
            if half == 0 and h == 1:
                # prefetch the (resident) FFN1 weights during attention
                for k in range(KX):
                    nc.sync.dma_start(w1_sb[:, k, :],
                                      d["w1T"][k * P:(k + 1) * P, :])

        # out-projection for this half -> bf16 partials -> ReduceScatter
        # (psum comes from the lg pool: lg is idle between halves)
        if True:
            for m in range(D // P):
                ps = ps_lg.tile([P, QH], F32, name="lg", tag="lg")
                for h in range(NH):
                    nc.tensor.matmul(ps, ow_sb[:, h, m * P:(m + 1) * P],
                                     att64[h], start=(h == 0), stop=(h == NH - 1))
                ot = pool_osb.tile([P, QH], BF16, name="ot", tag="ot")
                nc.scalar.copy(ot, ps)
                for r2 in range(4):
                    nc.sync.dma_start(
                        rs_in[half][r2, m * P:(m + 1) * P, :],
                        ot[:, r2 * 128:(r2 + 1) * 128])
        nc.gpsimd.collective_compute(
            "ReduceScatter",
            ALU.add,
            replica_groups=REPLICA_GROUPS,
            ins=[rs_in[half].opt()],
            outs=[rs_out[half].opt()],
        )

    for p in (pool_ps_v, ps_dum, ps_nbc, ps_att, ps_lg,
              pool_osb, pool_ow, pool_rrow, pool_nbc, pool_a64, pool_pr,
              pool_eg, pool_mask, pool_z, pool_adj, pool_wq):
        p.close()
    pool_qkv.close()

    # =================== residual + FFN (sequence-parallel) ===================
    with tc.tile_pool(name="ffn", bufs=1) as pool_f, \
         tc.tile_pool(name="w2s", bufs=8) as pool_w2, \
         tc.tile_pool(name="rsld", bufs=4) as pool_rsld, \
         tc.tile_pool(name="yout", bufs=3) as pool_yo, \
         tc.tile_pool(name="ps_g", bufs=1, space="PSUM") as ps_g, \
         tc.tile_pool(name="ps_tp", bufs=2, space="PSUM") as ps_tp:

        y1T = pool_f.tile([P, KX, QS], BF16, tag="y1T")
        y1tm = pool_f.tile([P, 2, D], BF16, tag="y1tm")
        g_tm = pool_f.tile([P, 2, FF], BF16, tag="g_tm")
        g_ff = pool_f.tile([P, FF // P, 2, P], BF16, tag="g_ff")
        adjy = pool_f.tile([2, QS], F32R, tag="adjy")      # [my ; stdy]
        adjf_w = pool_f.tile([2, FF], F32R, tag="adjf_w")
        nc.sync.dma_start(adjf_w, d["adjf"])
        b2r_sb = pool_f.tile([1, D], F32R, tag="b2r_sb")
        nc.sync.dma_start(b2r_sb, d["b2r"])
        ident_sb = pool_f.tile([P, P], BF16, tag="ident_sb")
        nc.sync.dma_start(ident_sb, d["ident"])
        ry_col = pool_f.tile([P, 2], F32, tag="ry_col")    # rstd_y token-striped

        # y1 = RS(out-proj partials) + x_slice + out_b   (feature-major).
        # half-0's shard assembles while RS(half 1) is still in flight.
        for hf in range(2):
            csl = slice(hf * 128, hf * 128 + 128)
            for m in range(KX):
                rst = pool_rsld.tile([P, 128], BF16, name="rst", tag="rst")
                nc.sync.dma_start(rst, rs_out[hf][m * P:(m + 1) * P, :])
                xqt = pool_rsld.tile([P, 128], F32, name="xqt", tag="xqt")
                nc.sync.dma_start(xqt, d["xq"][m * P:(m + 1) * P, csl])
                nc.vector.scalar_tensor_tensor(
                    out=y1T[:, m, csl], in0=rst, scalar=outb_col[:, m:m + 1],
                    in1=xqt, op0=ALU.add, op1=ALU.add)

        # y1 LN stats (fused chain) + 1/std to token-major columns
        with tc.tile_pool(name="ps_yst", bufs=1, space="PSUM") as ps_yst, \
             tc.tile_pool(name="ysq", bufs=2) as pool_ysq, \
             tc.tile_pool(name="yrows", bufs=1) as pool_yr:
            ps_sum = ps_yst.tile([1, QS], F32, name="ps_sum2", tag="ps_sum2")
            ps_ssq = ps_yst.tile([1, QS], F32, name="ps_ssq2", tag="ps_ssq2")
            for k in range(KX):
                nc.tensor.matmul(ps_sum, ones_col, y1T[:, k, :],
                                 start=(k == 0), stop=(k == KX - 1))
                sq = pool_ysq.tile([P, QS], BF16, name="ysq_t", tag="ysq_t")
                nc.vector.tensor_mul(sq, y1T[:, k, :], y1T[:, k, :])
                nc.tensor.matmul(ps_ssq, ones_col, sq,
                                 start=(k == 0), stop=(k == KX - 1))
            m2r = pool_yr.tile([1, QS], F32, tag="m2r")
            e2r = pool_yr.tile([1, QS], F32, tag="e2r")
            invr = pool_yr.tile([1, QS], F32R, tag="invr")
            ryr = pool_yr.tile([1, QS], F32, tag="ryr")
            nc.vector.tensor_scalar_mul(adjy[0:1, :], ps_sum, 1.0 / D)
            nc.vector.tensor_mul(m2r, adjy[0:1, :].bitcast(F32),
                                 adjy[0:1, :].bitcast(F32))
            nc.vector.scalar_tensor_tensor(out=e2r, in0=ps_ssq, scalar=1.0 / D,
                                           in1=m2r, op0=ALU.mult,
                                           op1=ALU.subtract)
            nc.scalar.activation(invr, e2r, AF.Sqrt, bias=eps_t[0:1])
            nc.vector.reciprocal_approx_fast(ryr, invr.bitcast(F32))
            nc.sync.dma_start(adjy[1:2, :], invr)
            nc.sync.dma_start(rs_scr[0:1, 0:QS], ryr)
        nc.sync.dma_start(
            ry_col, rs_scr[0:1, 0:QS].rearrange("a (i p) -> (a p) i", p=P))

        # y1 token-major (for the final residual add): 16 PE transposes
        for qt in range(2):
            for m in range(KX):
                tp = ps_tp.tile([P, 1024], BF16, name="tp", tag="tp")
                nc.tensor.transpose(tp[:, 0:P], y1T[:, m, qt * P:(qt + 1) * P],
                                    ident_sb)
                nc.scalar.copy(y1tm[:, qt, m * P:(m + 1) * P], tp[:, 0:P])

        # FFN1 operand-swapped: stationary y1 tiles, w1 rows moving (N=512).
        # gelu(ry * (raw + ADJ)) via the ACT scale port (per-token 1/std is
        # per-partition in token-major).
        for qt in range(2):
            qtl = slice(qt * P, (qt + 1) * P)
            for fg in range(2):
                pgs = [ps_g.tile([P, 512], F32, name=f"ps_g{i}", tag=f"ps_g{i}")
                       for i in range(4)]
                for k in range(KX):
                    for f4 in range(4):
                        fsl = slice(fg * 2048 + f4 * 512,
                                    fg * 2048 + f4 * 512 + 512)
                        nc.tensor.matmul(pgs[f4], y1T[:, k, qtl],
                                         w1_sb[:, k, fsl],
                                         start=(k == 0), stop=False)
                for f4 in range(4):
                    fsl = slice(fg * 2048 + f4 * 512, fg * 2048 + f4 * 512 + 512)
                    nc.tensor.matmul(pgs[f4], _r(adjy[:, qtl]),
                                     _r(adjf_w[:, fsl]), start=False, stop=True)
                    nc.scalar.activation(g_tm[:, qt, fsl], pgs[f4], AF.Gelu,
                                         scale=ry_col[:, qt:qt + 1])
                    # transpose back to ff-major right away so the copies
                    # overlap the FFN1 matmul stream instead of serializing
                    for j in range(fsl.start // P, fsl.stop // P):
                        tp = ps_tp.tile([P, 1024], BF16, name="tp", tag="tp")
                        nc.tensor.transpose(
                            tp[:, 0:P], g_tm[:, qt, j * P:(j + 1) * P],
                            ident_sb)
                        nc.scalar.copy(g_ff[:, j, qt, :], tp[:, 0:P])

        # FFN2 operand-swapped: stationary g tiles, w2 rows moving (N=512);
        # bias joins as a K=1 matmul; output is token-major.
        pys = [[ps_g.tile([P, 512], F32, name=f"ps_g{2 * qt + dc}",
                          tag=f"ps_g{2 * qt + dc}") for dc in range(2)]
               for qt in range(2)]
        for j in range(FF // P):
            w2b = pool_w2.tile([P, D], BF16, name="w2b", tag="w2b")
            nc.sync.dma_start(w2b, d["w2T"][j * P:(j + 1) * P, :])
            for qt in range(2):
                for dc in range(2):
                    nc.tensor.matmul(pys[qt][dc], g_ff[:, j, qt, :],
                                     w2b[:, dc * 512:(dc + 1) * 512],
                                     start=(j == 0), stop=False)
        for qt in range(2):
            for dc in range(2):
                dsl = slice(dc * 512, (dc + 1) * 512)
                nc.tensor.matmul(pys[qt][dc], _r(ones32[0:1, 0:P]),
                                 b2r_sb[0:1, dsl], start=False, stop=True)
                yt = pool_yo.tile([P, 512], F32, name="yt", tag="yt")
                nc.vector.tensor_add(yt, pys[qt][dc], y1tm[:, qt, dsl])
                nc.sync.dma_start(d["out"][qt * P:(qt + 1) * P, dsl], yt)

    pool_w1.close()
    const.close()
    dram.close()


def host_prep(inputs):
    """Fold layernorm gains/biases into weights; build the 8 per-core shards."""
    import ml_dtypes
    bf16 = ml_dtypes.bfloat16
    f32 = np.float32
    x = np.asarray(inputs["x"], f32)
    z = np.asarray(inputs["z"], f32)
    mask = np.asarray(inputs["attn_mask"], f32)
    gq = np.asarray(inputs["gq"], np.float64)
    bq = np.asarray(inputs["bq"], np.float64)
    gkv = np.asarray(inputs["gkv"], np.float64)
    bkv = np.asarray(inputs["bkv"], np.float64)
    gff = np.asarray(inputs["gff"], np.float64)
    bff = np.asarray(inputs["bff"], np.float64)
    ipw = np.asarray(inputs["in_proj_w"], np.float64)
    ipb = np.asarray(inputs["in_proj_b"], np.float64)
    out_w = np.asarray(inputs["out_w"], f32)
    out_b = np.asarray(inputs["out_b"], f32)
    w1 = np.asarray(inputs["w1"], np.float64)
    b1 = np.asarray(inputs["b1"], np.float64)
    w2 = np.asarray(inputs["w2"], f32)
    b2 = np.asarray(inputs["b2"], f32)

    wq, wk, wv = ipw[:D], ipw[D:2 * D], ipw[2 * D:]
    pq, pk, pv = ipb[:D], ipb[D:2 * D], ipb[2 * D:]
    scale = 1.0 / np.sqrt(HD)
    wq2 = (wq * gq[None, :]) * scale
    pq2 = (wq @ bq + pq) * scale
    wk2 = wk * gkv[None, :]
    pk2 = wk @ bkv + pk
    wv2 = wv * gkv[None, :]
    pv2 = wv @ bkv + pv
    w12 = w1 * gff[None, :]
    b12 = w1 @ bff + b1

    w1T = np.ascontiguousarray(w12.T.astype(f32)).astype(bf16)       # (D, FF)
    adjf = np.ascontiguousarray(
        np.stack([-w12.sum(1), b12]).astype(f32))                    # (2, FF)
    w2T = np.ascontiguousarray(w2.T).astype(bf16)                    # (FF, D)

    def pack_kxf(wT):  # (D, FC) -> (P, D//P, FC)
        return np.ascontiguousarray(
            wT.reshape(KX, P, FC).transpose(1, 0, 2)).astype(bf16)

    in_maps = []
    for c in range(NCORES):
        b, hg = c // 4, c % 4
        fs = slice(FC * hg, FC * hg + FC)
        # owned queries: [512*half + 128*hg, +128) for half in {0, 1}
        qown = np.concatenate([np.arange(128 * hg, 128 * hg + 128),
                               np.arange(512 + 128 * hg, 512 + 128 * hg + 128)])
        xTb = np.ascontiguousarray(x[b].T)                           # (D, Q)
        em = np.exp(mask[16 * b + NH * hg:16 * b + NH * hg + NH])    # (NH, Q, S)
        emT = em.transpose(0, 2, 1)                                  # (NH, S, Q)
        emp = np.ascontiguousarray(
            emT.reshape(NH, S // P, P, 2, QH).transpose(0, 3, 1, 2, 4)
        ).astype(bf16)                                               # (NH,2,16,P,QH)
        in_maps.append({
            "xT": xTb.astype(bf16),
            "zT": np.ascontiguousarray(z[b].T).astype(bf16),
            "emaskT": emp,
            "wqT": pack_kxf(np.ascontiguousarray(wq2[fs].T.astype(f32))),
            "wkT": pack_kxf(np.ascontiguousarray(wk2[fs].T.astype(f32))),
            "wvT": pack_kxf(np.ascontiguousarray(wv2[fs].T.astype(f32))),
            "adjq": np.ascontiguousarray(
                np.stack([-wq2[fs].sum(1), pq2[fs]]).astype(f32)),
            "adjk": np.ascontiguousarray(
                np.stack([-wk2[fs].sum(1), pk2[fs]]).astype(f32)),
            "adjv": np.ascontiguousarray(
                np.stack([-wv2[fs].sum(1), pv2[fs]]).astype(f32)),
            "owh": np.ascontiguousarray(
                out_w[:, fs].T.reshape(NH, HD, D).transpose(1, 0, 2)
            ).astype(bf16),
            "outb": out_b,
            "xq": np.ascontiguousarray(xTb[:, qown]),
            "w1T": w1T,
            "adjf": adjf,
            "w2T": w2T,
            "b2r": np.ascontiguousarray(b2.reshape(1, D)),
            "ident": np.eye(P, dtype=f32).astype(bf16),
        })
    return in_maps


_NC_CACHE = None


def kernel(**inputs) -> np.ndarray:
    global _NC_CACHE, LAST_RESULT
    from concourse.bass_utils import run_bass_kernel_spmd

    in_maps = host_prep(inputs)
    if _NC_CACHE is None:
        _NC_CACHE = build_nc()
    res = run_bass_kernel_spmd(
        _NC_CACHE, in_maps, core_ids=list(range(NCORES)),
        trace=bool(os.environ.get("BASS_TRACE")),
    )
    LAST_RESULT = res
    out = np.empty((B, Q, D), np.float32)
    for c in range(NCORES):
        b, hg = c // 4, c % 4
        o = res.results[c]["out"]                       # (QS, D) token-major
        out[b, 128 * hg:128 * hg + 128, :] = o[0:128]
        out[b, 512 + 128 * hg:512 + 128 * hg + 128, :] = o[128:256]
    return out


# revision 42
# speedup vs baseline: 1.0987x; 1.0987x over previous
# Bass/Tile TRN2 kernel for nn_BiasedCrossDecoderLayer (dense cross-attention
# transformer decoder layer), SPMD over 8 NeuronCores.
#
# Sharding: core c -> batch b = c//4, head-group hg = c%4 (4 of 16 heads =
# 256 of 1024 qkv feature dims).  Attention is head-parallel; the
# out-projection produces partial sums which are ReduceScattered (along the
# query axis) within each 4-core batch group; the FFN then runs
# sequence-parallel on each core's 256-query slice with the full 4096 hidden.
# Each core ends up owning queries [512H + 128r, +128) for H in {0,1}.
#
# v3 highlights:
#  - All matmul operands bf16 (same PE rate, half DMA/SBUF, 2x DVE); LN stat
#    rows fp32r.  softmax is exp(logits) * exp(mask) with exp(mask) from host.
#  - Attention in two query-halves, each half's out-proj feeds a bf16
#    ReduceScatter immediately so RS(half0) hides under attention(half1).
#  - The V projection is braided into attention(half0, head0)'s instruction
#    stream and small filler matmuls pad the other head loops: the PE never
#    idles, which keeps the HAM clock gate at 2.4 GHz (idle-y streams measure
#    ~1.1 GHz).
#  - FFN runs operand-swapped: stationary activations, weights as the moving
#    rhs (N=512), so LDWEIGHTS is off the critical path.  FFN1 emits gelu via
#    the ACT scale port (per-token 1/std lives on partitions in token-major);
#    g is PE-transposed back to ff-major for FFN2; the final bias is a K=1
#    matmul appended to the FFN2 accumulation.  Output leaves token-major.
#  - reciprocal_approx_fast (18 bits) instead of the 3.3us DVE reciprocal.
#
# LayerNorm folding (host):  q = LN(x;g,b) @ Wq.T + pq == LN0(x) @ Wq'.T + b'
# with Wq' = wq*g (attention 1/sqrt(hd) folded in), b' = wq@b + pq.  In
# feature-major layout qT = rB * (Wq' @ xT_raw + ADJ) where
# ADJ[o,t] = -rowsum(Wq')[o]*m[t] + b'[o]*std[t] is a K=2 matmul appended to
# the PSUM accumulation.  V is produced token-major with the analogous fold.
# The softmax denominator is a ones-column appended to V (M=65 matmul).

import os
import sys

import numpy as np

sys.path.insert(0, "/opt/trn_rl_repo")

import concourse.bass as bass  # noqa: E402
import concourse.mybir as mybir  # noqa: E402
import concourse.tile as tile  # noqa: E402
from concourse import bacc  # noqa: E402

F32 = mybir.dt.float32
F32R = mybir.dt.float32r
BF16 = mybir.dt.bfloat16
AF = mybir.ActivationFunctionType
ALU = mybir.AluOpType

B, Q, S, D, H = 2, 1024, 2048, 1024, 16
HD = D // H       # 64
FF = 4 * D
EPS = 1e-5
NCORES = 8
NH = 4            # heads per core
FC = NH * HD      # 256 qkv feature dims per core
QS = 256          # owned queries per core (two 128-slices, one per Q-half)
P = 128
KX = D // P       # 8 k-tiles over the model dim
QH = Q // 2       # 512-query attention half

REPLICA_GROUPS = [[0, 1, 2, 3], [4, 5, 6, 7]]

LAST_RESULT = None  # BassKernelResults of the most recent run (for test.py)


def _r(ap):
    """View an fp32 AP as float32r for full-rate PE matmuls."""
    return ap.bitcast(F32R)


def build_nc():
    nc = bacc.Bacc(
        "TRN2",
        target_bir_lowering=False,
        debug=False,
        num_devices=NCORES,
        name="biased_cross_decoder",
    )

    # ---- DRAM I/O (per-core shards; same program on all cores) ----
    d = {}
    d["xT"] = nc.dram_tensor("xT", [D, Q], BF16, kind="ExternalInput").ap()
    d["zT"] = nc.dram_tensor("zT", [D, S], BF16, kind="ExternalInput").ap()
    d["emaskT"] = nc.dram_tensor(
        "emaskT", [NH, 2, S // P, P, QH], BF16, kind="ExternalInput").ap()
    d["wqT"] = nc.dram_tensor("wqT", [P, KX, FC], BF16, kind="ExternalInput").ap()
    d["wkT"] = nc.dram_tensor("wkT", [P, KX, FC], BF16, kind="ExternalInput").ap()
    d["wvT"] = nc.dram_tensor("wvT", [P, KX, FC], BF16, kind="ExternalInput").ap()
    d["adjq"] = nc.dram_tensor("adjq", [2, FC], F32R, kind="ExternalInput").ap()
    d["adjk"] = nc.dram_tensor("adjk", [2, FC], F32R, kind="ExternalInput").ap()
    d["adjv"] = nc.dram_tensor("adjv", [2, FC], F32R, kind="ExternalInput").ap()
    d["owh"] = nc.dram_tensor("owh", [HD, NH, D], BF16, kind="ExternalInput").ap()
    d["outb"] = nc.dram_tensor("outb", [D], F32, kind="ExternalInput").ap()
    d["xq"] = nc.dram_tensor("xq", [D, QS], F32, kind="ExternalInput").ap()
    d["w1T"] = nc.dram_tensor("w1T", [D, FF], BF16, kind="ExternalInput").ap()
    d["adjf"] = nc.dram_tensor("adjf", [2, FF], F32R, kind="ExternalInput").ap()
    d["w2T"] = nc.dram_tensor("w2T", [FF, D], BF16, kind="ExternalInput").ap()
    d["b2r"] = nc.dram_tensor("b2r", [1, D], F32R, kind="ExternalInput").ap()
    d["ident"] = nc.dram_tensor("ident", [P, P], BF16, kind="ExternalInput").ap()
    d["out"] = nc.dram_tensor("out", [QS, D], F32, kind="ExternalOutput").ap()

    with tile.TileContext(nc) as tc:
        build_tile_program(tc, nc, d)
    nc.compile()   # bacc passes: wait splitting, ldweights fusion, reg alloc
    return nc


class _Pool:
    """Keeps the tile_pool context manager alive; allows explicit close."""

    def __init__(self, cm):
        self._cm = cm
        self.pool = cm.__enter__()

    def tile(self, *a, **kw):
        kw.setdefault("name", kw.get("tag") or "t")
        return self.pool.tile(*a, **kw)

    def close(self):
        self._cm.__exit__(None, None, None)


def build_tile_program(tc, nc, d):
    # ---------------- persistent constants ----------------
    const = _Pool(tc.tile_pool(name="const", bufs=1))
    dram = _Pool(tc.tile_pool(name="dram", bufs=1, space="DRAM"))

    ones_sb = const.tile([P, P], BF16, tag="ones_sb")
    nc.vector.memset(ones_sb, 1.0)
    ones_col = ones_sb[:, 0:1]                  # lhsT for column sums
    ones32 = const.tile([P, P], F32, tag="ones32")   # f32r lhsT for broadcasts
    nc.vector.memset(ones32, 1.0)
    eps_t = const.tile([1, 1], F32, tag="eps")
    nc.vector.memset(eps_t, EPS)
    outb_col = const.tile([P, KX], F32, tag="outb_col")
    nc.sync.dma_start(outb_col, d["outb"].rearrange("(o p) -> p o", p=P))
    rz_col = const.tile([P, S // P], F32, tag="rz_col")     # rstd_z token-striped

    rs_scr = dram.tile([1, S], F32, tag="rs_scr")           # row restripe bounce
    rs_in = [dram.tile([4, D, QH // 4], BF16, name=f"rs_in{i}", tag=f"rs_in{i}")
             for i in range(2)]
    rs_out = [dram.tile([D, QH // 4], BF16, name=f"rs_out{i}", tag=f"rs_out{i}")
              for i in range(2)]

    # ---------------- long-lived right-side pools ----------------
    pool_w1 = _Pool(tc.tile_pool(name="w1", bufs=1, side="right"))
    w1_sb = pool_w1.tile([P, KX, FF], BF16, tag="w1_sb")    # loaded in attn phase
    pool_qkv = _Pool(tc.tile_pool(name="qkv", bufs=1, side="right"))
    qT = pool_qkv.tile([P, FC // P, Q], BF16, tag="qT")   # includes 1/8 scale
    kT = pool_qkv.tile([P, FC // P, S], BF16, tag="kT")
    v_sb = pool_qkv.tile([P, S // P, NH, HD + 1], BF16, tag="v_sb")

    # ---------------- front-phase scratch pools (left stack) ----------------
    # stack order (bottom->top): wq, adj, pz | rows, sq, rb, px
    # the right of the bar closes at front-phase end; the left survives the
    # attention braid (v-proj needs zT/adjz/wv) and closes after attention.
    pool_wq = _Pool(tc.tile_pool(name="wq", bufs=1))
    pool_adj = _Pool(tc.tile_pool(name="adj", bufs=1))
    pool_z = _Pool(tc.tile_pool(name="pz", bufs=1))
    pool_rows = _Pool(tc.tile_pool(name="rows", bufs=1))
    pool_sq = _Pool(tc.tile_pool(name="sq", bufs=3))
    pool_rb = _Pool(tc.tile_pool(name="rb", bufs=6))      # 1/std row-broadcasts
    pool_x = _Pool(tc.tile_pool(name="px", bufs=1))

    adjx = pool_adj.tile([2, Q], F32R, tag="adjx")        # [mx ; stdx]
    adjz = pool_adj.tile([2, S], F32R, tag="adjz")        # [mz ; stdz]

    zT = pool_z.tile([P, KX, S], BF16, tag="zT")
    xT = pool_x.tile([P, KX, Q], BF16, tag="xT")
    for ch in range(2):
        for k in range(KX):
            nc.sync.dma_start(xT[:, k, ch * 512:(ch + 1) * 512],
                              d["xT"][k * P:(k + 1) * P, ch * 512:(ch + 1) * 512])
    for ch in range(4):
        for k in range(KX):
            nc.sync.dma_start(zT[:, k, ch * 512:(ch + 1) * 512],
                              d["zT"][k * P:(k + 1) * P, ch * 512:(ch + 1) * 512])

    def ln_chunk_stats(aT, sl, adj, ps_stats, scr=None):
        """LN stats for one 512-token chunk: adj[:, sl] = [mean ; std]; returns
        the 1/std broadcast tile [P, 512] (SBUF, f32)."""
        w = sl.stop - sl.start
        ps_sum = ps_stats.tile([1, w], F32, name="ps_sum", tag="ps_sum")
        ps_ssq = ps_stats.tile([1, w], F32, name="ps_ssq", tag="ps_ssq")
        for k in range(KX):
            nc.tensor.matmul(ps_sum, ones_col, aT[:, k, sl],
                             start=(k == 0), stop=(k == KX - 1))
        for k in range(KX):
            sq = pool_sq.tile([P, w], BF16, name="sq", tag="sq")
            nc.vector.tensor_mul(sq, aT[:, k, sl], aT[:, k, sl])
            nc.tensor.matmul(ps_ssq, ones_col, sq,
                             start=(k == 0), stop=(k == KX - 1))
        m2 = pool_rows.tile([1, w], F32, name="m2", tag="m2")
        e2 = pool_rows.tile([1, w], F32, name="e2", tag="e2")
        inv = pool_rows.tile([1, w], F32R, name="inv", tag="inv")
        rr = pool_rows.tile([1, w], F32, name="rr", tag="rr")
        rrr = pool_rows.tile([1, w], F32R, name="rrr", tag="rrr")
        nc.vector.tensor_scalar_mul(adj[0:1, sl], ps_sum, 1.0 / D)  # mean
        nc.vector.tensor_mul(m2, adj[0:1, sl].bitcast(F32), adj[0:1, sl].bitcast(F32))
        nc.vector.scalar_tensor_tensor(out=e2, in0=ps_ssq, scalar=1.0 / D,
                                       in1=m2, op0=ALU.mult, op1=ALU.subtract)
        nc.scalar.activation(inv, e2, AF.Sqrt, bias=eps_t[0:1])     # std
        nc.vector.reciprocal_approx_fast(rr, inv.bitcast(F32))
        nc.scalar.copy(rrr, rr)              # f32r-rounded copy for the PE
        nc.sync.dma_start(adj[1:2, sl], inv)  # cross-partition row move
        bcp = ps_stats.tile([P, w], F32, name="bcp", tag="bcp")
        nc.tensor.matmul(bcp, _r(ones32[0:1, :]), rrr)
        rB = pool_rb.tile([P, w], F32, name="rB", tag="rB")
        nc.scalar.copy(rB, bcp)
        if scr is not None:
            nc.sync.dma_start(scr[0:1, sl], rr)
        return rB

    # =============== front phase: x stats+qproj, z stats, k proj ===============
    ps_qk = _Pool(tc.tile_pool(name="ps_qk", bufs=1, space="PSUM"))
    ps_st = _Pool(tc.tile_pool(name="ps_st", bufs=1, space="PSUM"))

    wq_sb = pool_wq.tile([P, KX, FC], BF16, tag="wq_sb")
    nc.sync.dma_start(wq_sb, d["wqT"])
    wk_sb = pool_wq.tile([P, KX, FC], BF16, tag="wk_sb")
    nc.sync.dma_start(wk_sb, d["wkT"])
    wv_sb = pool_wq.tile([P, KX, FC], BF16, tag="wv_sb")
    nc.sync.dma_start(wv_sb, d["wvT"])
    adjq_w = pool_wq.tile([2, FC], F32R, tag="adjq_w")
    nc.sync.dma_start(adjq_w, d["adjq"])
    adjk_w = pool_wq.tile([2, FC], F32R, tag="adjk_w")
    nc.sync.dma_start(adjk_w, d["adjk"])
    adjv_w = pool_wq.tile([2, FC], F32R, tag="adjv_w")
    nc.sync.dma_start(adjv_w, d["adjv"])

    # x: stats + q projection, 512-token chunks
    for ch in range(2):
        sl = slice(ch * 512, (ch + 1) * 512)
        rxB = ln_chunk_stats(xT, sl, adjx, ps_st)
        pss = []
        for m in range(FC // P):
            ps = ps_qk.tile([P, 512], F32, name=f"ps_q{m}", tag=f"ps_q{m}")
            for k in range(KX):
                nc.tensor.matmul(ps, wq_sb[:, k, m * P:(m + 1) * P],
                                 xT[:, k, sl], start=(k == 0), stop=False)
            pss.append(ps)
        for m in range(FC // P):
            nc.tensor.matmul(pss[m], _r(adjq_w[:, m * P:(m + 1) * P]),
                             _r(adjx[:, sl]), start=False, stop=True)
            nc.vector.tensor_mul(qT[:, m, sl], pss[m], rxB)

    # z: stats for all chunks, then k projection
    rzBs = []
    for ch in range(4):
        sl = slice(ch * 512, (ch + 1) * 512)
        rzBs.append(ln_chunk_stats(zT, sl, adjz, ps_st, scr=rs_scr))
    # restripe 1/std_z to token-major columns for the v normalize
    nc.sync.dma_start(rz_col, rs_scr.rearrange("a (i p) -> (a p) i", p=P))

    for ch in range(4):
        sl = slice(ch * 512, (ch + 1) * 512)
        for m in range(FC // P):
            ps = ps_qk.tile([P, 512], F32, name=f"ps_q{m}", tag=f"ps_q{m}")
            for k in range(KX):
                nc.tensor.matmul(ps, wk_sb[:, k, m * P:(m + 1) * P],
                                 zT[:, k, sl], start=(k == 0), stop=False)
            nc.tensor.matmul(ps, _r(adjk_w[:, m * P:(m + 1) * P]),
                             _r(adjz[:, sl]), start=False, stop=True)
            nc.vector.tensor_mul(kT[:, m, sl], ps, rzBs[ch])

    # front-phase scratch no longer needed; free before attention pools open
    pool_x.close()
    pool_rb.close()
    pool_sq.close()
    pool_rows.close()
    ps_st.close()
    ps_qk.close()


    # softmax-denominator ones column
    nc.vector.memset(v_sb[:, :, :, HD:HD + 1], 1.0)

    def v_proj(t):
        """One token-tile of the V projection (braided into attention h0)."""
        ps = pool_ps_v.tile([P, FC], F32, name="ps_v_t", tag="ps_v_t")
        for k in range(KX):
            nc.tensor.matmul(ps, zT[:, k, t * P:(t + 1) * P],
                             wv_sb[:, k, :], start=(k == 0), stop=False)
        nc.tensor.matmul(ps, _r(adjz[:, t * P:(t + 1) * P]), _r(adjv_w),
                         start=False, stop=True)
        nc.vector.tensor_scalar_mul(
            v_sb[:, t, :, 0:HD],
            ps.rearrange("p (h e) -> p h e", h=NH),
            rz_col[:, t:t + 1])

    # ============ attention + out-projection + RS, per query-half ============
    pool_mask = _Pool(tc.tile_pool(name="mask", bufs=4))
    pool_eg = _Pool(tc.tile_pool(name="eg", bufs=3))
    pool_pr = _Pool(tc.tile_pool(name="probs", bufs=4))
    pool_a64 = _Pool(tc.tile_pool(name="a64", bufs=2))
    pool_nbc = _Pool(tc.tile_pool(name="nbc", bufs=2))
    pool_rrow = _Pool(tc.tile_pool(name="rrow", bufs=2))
    pool_ow = _Pool(tc.tile_pool(name="ow", bufs=1))
    pool_osb = _Pool(tc.tile_pool(name="osb", bufs=3))
    ps_lg = _Pool(tc.tile_pool(name="ps_lg", bufs=2, space="PSUM"))
    ps_att = _Pool(tc.tile_pool(name="ps_att", bufs=2, space="PSUM"))
    ps_nbc = _Pool(tc.tile_pool(name="ps_nbc", bufs=1, space="PSUM"))
    pool_ps_v = _Pool(tc.tile_pool(name="ps_v", bufs=1, space="PSUM"))

    ow_sb = pool_ow.tile([HD, NH, D], BF16, tag="ow_sb")
    nc.sync.dma_start(ow_sb, d["owh"])
    # v projection in the dense (full-clock) stream, not the attention braid:
    # half-array attention matmuls run HAM-throttled at 1.2 GHz, so braided
    # v-proj work would pay 2x its cycles there
    for t in range(S // P):
        v_proj(t)

    for half in range(2):
        qsl = slice(half * QH, (half + 1) * QH)
        att64 = []
        for h in range(NH):
            ht, ho = h // 2, HD * (h % 2)
            att_ps = ps_att.tile([HD + 1, QH], F32, name="att_ps", tag="att_ps")
            prs = [None] * (S // P)
            for st in range(S // P):
                mk = pool_mask.tile([P, QH], BF16, name="mk", tag="mk")
                nc.sync.dma_start(mk, d["emaskT"][h, half, st])
                lg = ps_lg.tile([P, QH], F32, name="lg", tag="lg")
                nc.tensor.matmul(
                    lg, kT[ho:ho + HD, ht, st * P:(st + 1) * P],
                    qT[ho:ho + HD, ht, qsl])
                eg = pool_eg.tile([P, QH], BF16, name="eg", tag="eg")
                nc.scalar.activation(eg, lg, AF.Exp)
                pr = pool_pr.tile([P, QH], BF16, name="pr", tag="pr")
                nc.vector.tensor_mul(pr, eg, mk)
                prs[st] = pr
                # PV accumulation two steps behind: the exp+mul chain is
                # ~1.1us, more than one 2.4 GHz iteration period, so a
                # one-step pipeline still stalls the PE every iteration
                if st > 1:
                    nc.tensor.matmul(att_ps, v_sb[:, st - 2, h, :],
                                     prs[st - 2], start=(st == 2), stop=False)
                    prs[st - 2] = None

            for st in (S // P - 2, S // P - 1):
                nc.tensor.matmul(att_ps, v_sb[:, st, h, :], prs[st],
                                 start=False, stop=(st == S // P - 1))
            # normalize: att[0:64] * broadcast(1 / att[64])
            rr = pool_rrow.tile([HD + 1, QH], F32R, name="rr", tag="rr")
            with nc.allow_low_precision(reason="fp32r rounding of 1/sum"):
                nc.vector.reciprocal(rr[HD:HD + 1, :], att_ps[HD:HD + 1, :])
            bc = ps_nbc.tile([HD, QH], F32, name="bc2", tag="bc2")
            nc.tensor.matmul(bc, _r(ones32[HD:HD + 1, 0:HD]),
                             rr[HD:HD + 1, :])
            nbc = pool_nbc.tile([HD, QH], F32, name="nbc_t", tag="nbc_t")
            nc.scalar.copy(nbc, bc)
            a64 = pool_a64.tile([HD, QH], BF16, name=f"a64_{h}", tag=f"a64_{h}")
            nc.vector.tensor_mul(a64, att_ps[0:HD, :], nbc)
            att64.append(a64)
            if half == 0 and h == 1:
                # prefetch the (resident) FFN1 weights during attention
                for k in range(KX):
                    nc.sync.dma_start(w1_sb[:, k, :],
                                      d["w1T"][k * P:(k + 1) * P, :])

        # out-projection for this half -> bf16 partials -> ReduceScatter
        # (psum comes from the lg pool: lg is idle between halves)
        if True:
            for m in range(D // P):
                ps = ps_lg.tile([P, QH], F32, name="lg", tag="lg")
                for h in range(NH):
                    nc.tensor.matmul(ps, ow_sb[:, h, m * P:(m + 1) * P],
                                     att64[h], start=(h == 0), stop=(h == NH - 1))
                ot = pool_osb.tile([P, QH], BF16, name="ot", tag="ot")
                nc.scalar.copy(ot, ps)
                for r2 in range(4):
                    nc.sync.dma_start(
                        rs_in[half][r2, m * P:(m + 1) * P, :],
                        ot[:, r2 * 128:(r2 + 1) * 128])
        nc.gpsimd.collective_compute(
            "ReduceScatter",
            ALU.add,
            replica_groups=REPLICA_GROUPS,
            ins=[rs_in[half].opt()],
            outs=[rs_out[half].opt()],
        )

    for p in (pool_ps_v, ps_nbc, ps_att, ps_lg,
              pool_osb, pool_ow, pool_rrow, pool_nbc, pool_a64, pool_pr,
              pool_eg, pool_mask, pool_z, pool_adj, pool_wq):
        p.close()
    pool_qkv.close()

    # =================== residual + FFN (sequence-parallel) ===================
    with tc.tile_pool(name="ffn", bufs=1) as pool_f, \
         tc.tile_pool(name="w2s", bufs=8) as pool_w2, \
         tc.tile_pool(name="rsld", bufs=4) as pool_rsld, \
         tc.tile_pool(name="yout", bufs=3) as pool_yo, \
         tc.tile_pool(name="ps_g", bufs=1, space="PSUM") as ps_g, \
         tc.tile_pool(name="ps_tp", bufs=2, space="PSUM") as ps_tp:

        y1T = pool_f.tile([P, KX, QS], BF16, tag="y1T")
        y1tm = pool_f.tile([P, 2, D], BF16, tag="y1tm")
        g_tm = pool_f.tile([P, 2, FF], BF16, tag="g_tm")
        g_ff = pool_f.tile([P, FF // P, 2, P], BF16, tag="g_ff")
        adjy = pool_f.tile([2, QS], F32R, tag="adjy")      # [my ; stdy]
        adjf_w = pool_f.tile([2, FF], F32R, tag="adjf_w")
        nc.sync.dma_start(adjf_w, d["adjf"])
        b2r_sb = pool_f.tile([1, D], F32R, tag="b2r_sb")
        nc.sync.dma_start(b2r_sb, d["b2r"])
        ident_sb = pool_f.tile([P, P], BF16, tag="ident_sb")
        nc.sync.dma_start(ident_sb, d["ident"])
        ry_col = pool_f.tile([P, 2], F32, tag="ry_col")    # rstd_y token-striped

        # y1 = RS(out-proj partials) + x_slice + out_b   (feature-major).
        # half-0's shard assembles while RS(half 1) is still in flight.
        for hf in range(2):
            csl = slice(hf * 128, hf * 128 + 128)
            for m in range(KX):
                rst = pool_rsld.tile([P, 128], BF16, name="rst", tag="rst")
                nc.sync.dma_start(rst, rs_out[hf][m * P:(m + 1) * P, :])
                xqt = pool_rsld.tile([P, 128], F32, name="xqt", tag="xqt")
                nc.sync.dma_start(xqt, d["xq"][m * P:(m + 1) * P, csl])
                nc.vector.scalar_tensor_tensor(
                    out=y1T[:, m, csl], in0=rst, scalar=outb_col[:, m:m + 1],
                    in1=xqt, op0=ALU.add, op1=ALU.add)

        # y1 LN stats (fused chain) + 1/std to token-major columns
        with tc.tile_pool(name="ps_yst", bufs=1, space="PSUM") as ps_yst, \
             tc.tile_pool(name="ysq", bufs=2) as pool_ysq, \
             tc.tile_pool(name="yrows", bufs=1) as pool_yr:
            ps_sum = ps_yst.tile([1, QS], F32, name="ps_sum2", tag="ps_sum2")
            ps_ssq = ps_yst.tile([1, QS], F32, name="ps_ssq2", tag="ps_ssq2")
            for k in range(KX):
                nc.tensor.matmul(ps_sum, ones_col, y1T[:, k, :],
                                 start=(k == 0), stop=(k == KX - 1))
                sq = pool_ysq.tile([P, QS], BF16, name="ysq_t", tag="ysq_t")
                nc.vector.tensor_mul(sq, y1T[:, k, :], y1T[:, k, :])
                nc.tensor.matmul(ps_ssq, ones_col, sq,
                                 start=(k == 0), stop=(k == KX - 1))
            m2r = pool_yr.tile([1, QS], F32, tag="m2r")
            e2r = pool_yr.tile([1, QS], F32, tag="e2r")
            invr = pool_yr.tile([1, QS], F32R, tag="invr")
            ryr = pool_yr.tile([1, QS], F32, tag="ryr")
            nc.vector.tensor_scalar_mul(adjy[0:1, :], ps_sum, 1.0 / D)
            nc.vector.tensor_mul(m2r, adjy[0:1, :].bitcast(F32),
                                 adjy[0:1, :].bitcast(F32))
            nc.vector.scalar_tensor_tensor(out=e2r, in0=ps_ssq, scalar=1.0 / D,
                                           in1=m2r, op0=ALU.mult,
                                           op1=ALU.subtract)
            nc.scalar.activation(invr, e2r, AF.Sqrt, bias=eps_t[0:1])
            nc.vector.reciprocal_approx_fast(ryr, invr.bitcast(F32))
            nc.sync.dma_start(adjy[1:2, :], invr)
            nc.sync.dma_start(rs_scr[0:1, 0:QS], ryr)
        nc.sync.dma_start(
            ry_col, rs_scr[0:1, 0:QS].rearrange("a (i p) -> (a p) i", p=P))

        # y1 token-major (for the final residual add): 16 PE transposes
        for qt in range(2):
            for m in range(KX):
                tp = ps_tp.tile([P, 1024], BF16, name="tp", tag="tp")
                nc.tensor.transpose(tp[:, 0:P], y1T[:, m, qt * P:(qt + 1) * P],
                                    ident_sb)
                nc.scalar.copy(y1tm[:, qt, m * P:(m + 1) * P], tp[:, 0:P])

        # FFN1 operand-swapped: stationary y1 tiles, w1 rows moving (N=512).
        # gelu(ry * (raw + ADJ)) via the ACT scale port (per-token 1/std is
        # per-partition in token-major).
        for qt in range(2):
            qtl = slice(qt * P, (qt + 1) * P)
            for fg in range(2):
                pgs = [ps_g.tile([P, 512], F32, name=f"ps_g{i}", tag=f"ps_g{i}")
                       for i in range(4)]
                for k in range(KX):
                    for f4 in range(4):
                        fsl = slice(fg * 2048 + f4 * 512,
                                    fg * 2048 + f4 * 512 + 512)
                        nc.tensor.matmul(pgs[f4], y1T[:, k, qtl],
                                         w1_sb[:, k, fsl],
                                         start=(k == 0), stop=False)
                for f4 in range(4):
                    fsl = slice(fg * 2048 + f4 * 512, fg * 2048 + f4 * 512 + 512)
                    nc.tensor.matmul(pgs[f4], _r(adjy[:, qtl]),
                                     _r(adjf_w[:, fsl]), start=False, stop=True)
                    nc.scalar.activation(g_tm[:, qt, fsl], pgs[f4], AF.Gelu,
                                         scale=ry_col[:, qt:qt + 1])
                    # transpose back to ff-major right away so the copies
                    # overlap the FFN1 matmul stream instead of serializing
                    for j in range(fsl.start // P, fsl.stop // P):
                        tp = ps_tp.tile([P, 1024], BF16, name="tp", tag="tp")
                        nc.tensor.transpose(
                            tp[:, 0:P], g_tm[:, qt, j * P:(j + 1) * P],
                            ident_sb)
                        nc.scalar.copy(g_ff[:, j, qt, :], tp[:, 0:P])

        # FFN2 operand-swapped: stationary g tiles, w2 rows moving (N=512);
        # bias joins as a K=1 matmul; output is token-major.
        pys = [[ps_g.tile([P, 512], F32, name=f"ps_g{2 * qt + dc}",
                          tag=f"ps_g{2 * qt + dc}") for dc in range(2)]
               for qt in range(2)]
        for j in range(FF // P):
            w2b = pool_w2.tile([P, D], BF16, name="w2b", tag="w2b")
            nc.sync.dma_start(w2b, d["w2T"][j * P:(j + 1) * P, :])
            for qt in range(2):
                for dc in range(2):
                    nc.tensor.matmul(pys[qt][dc], g_ff[:, j, qt, :],
                                     w2b[:, dc * 512:(dc + 1) * 512],
                                     start=(j == 0), stop=False)
        for qt in range(2):
            for dc in range(2):
                dsl = slice(dc * 512, (dc + 1) * 512)
                nc.tensor.matmul(pys[qt][dc], _r(ones32[0:1, 0:P]),
                                 b2r_sb[0:1, dsl], start=False, stop=True)
                yt = pool_yo.tile([P, 512], F32, name="yt", tag="yt")
                nc.vector.tensor_add(yt, pys[qt][dc], y1tm[:, qt, dsl])
                nc.sync.dma_start(d["out"][qt * P:(qt + 1) * P, dsl], yt)

    pool_w1.close()
    const.close()
    dram.close()


def host_prep(inputs):
    """Fold layernorm gains/biases into weights; build the 8 per-core shards."""
    import ml_dtypes
    bf16 = ml_dtypes.bfloat16
    f32 = np.float32
    x = np.asarray(inputs["x"], f32)
    z = np.asarray(inputs["z"], f32)
    mask = np.asarray(inputs["attn_mask"], f32)
    gq = np.asarray(inputs["gq"], np.float64)
    bq = np.asarray(inputs["bq"], np.float64)
    gkv = np.asarray(inputs["gkv"], np.float64)
    bkv = np.asarray(inputs["bkv"], np.float64)
    gff = np.asarray(inputs["gff"], np.float64)
    bff = np.asarray(inputs["bff"], np.float64)
    ipw = np.asarray(inputs["in_proj_w"], np.float64)
    ipb = np.asarray(inputs["in_proj_b"], np.float64)
    out_w = np.asarray(inputs["out_w"], f32)
    out_b = np.asarray(inputs["out_b"], f32)
    w1 = np.asarray(inputs["w1"], np.float64)
    b1 = np.asarray(inputs["b1"], np.float64)
    w2 = np.asarray(inputs["w2"], f32)
    b2 = np.asarray(inputs["b2"], f32)

    wq, wk, wv = ipw[:D], ipw[D:2 * D], ipw[2 * D:]
    pq, pk, pv = ipb[:D], ipb[D:2 * D], ipb[2 * D:]
    scale = 1.0 / np.sqrt(HD)
    wq2 = (wq * gq[None, :]) * scale
    pq2 = (wq @ bq + pq) * scale
    wk2 = wk * gkv[None, :]
    pk2 = wk @ bkv + pk
    wv2 = wv * gkv[None, :]
    pv2 = wv @ bkv + pv
    w12 = w1 * gff[None, :]
    b12 = w1 @ bff + b1

    w1T = np.ascontiguousarray(w12.T.astype(f32)).astype(bf16)       # (D, FF)
    adjf = np.ascontiguousarray(
        np.stack([-w12.sum(1), b12]).astype(f32))                    # (2, FF)
    w2T = np.ascontiguousarray(w2.T).astype(bf16)                    # (FF, D)

    def pack_kxf(wT):  # (D, FC) -> (P, D//P, FC)
        return np.ascontiguousarray(
            wT.reshape(KX, P, FC).transpose(1, 0, 2)).astype(bf16)

    in_maps = []
    for c in range(NCORES):
        b, hg = c // 4, c % 4
        fs = slice(FC * hg, FC * hg + FC)
        # owned queries: [512*half + 128*hg, +128) for half in {0, 1}
        qown = np.concatenate([np.arange(128 * hg, 128 * hg + 128),
                               np.arange(512 + 128 * hg, 512 + 128 * hg + 128)])
        xTb = np.ascontiguousarray(x[b].T)                           # (D, Q)
        em = np.exp(mask[16 * b + NH * hg:16 * b + NH * hg + NH])    # (NH, Q, S)
        emT = em.transpose(0, 2, 1)                                  # (NH, S, Q)
        emp = np.ascontiguousarray(
            emT.reshape(NH, S // P, P, 2, QH).transpose(0, 3, 1, 2, 4)
        ).astype(bf16)                                               # (NH,2,16,P,QH)
        in_maps.append({
            "xT": xTb.astype(bf16),
            "zT": np.ascontiguousarray(z[b].T).astype(bf16),
            "emaskT": emp,
            "wqT": pack_kxf(np.ascontiguousarray(wq2[fs].T.astype(f32))),
            "wkT": pack_kxf(np.ascontiguousarray(wk2[fs].T.astype(f32))),
            "wvT": pack_kxf(np.ascontiguousarray(wv2[fs].T.astype(f32))),
            "adjq": np.ascontiguousarray(
                np.stack([-wq2[fs].sum(1), pq2[fs]]).astype(f32)),
            "adjk": np.ascontiguousarray(
                np.stack([-wk2[fs].sum(1), pk2[fs]]).astype(f32)),
            "adjv": np.ascontiguousarray(
                np.stack([-wv2[fs].sum(1), pv2[fs]]).astype(f32)),
            "owh": np.ascontiguousarray(
                out_w[:, fs].T.reshape(NH, HD, D).transpose(1, 0, 2)
            ).astype(bf16),
            "outb": out_b,
            "xq": np.ascontiguousarray(xTb[:, qown]),
            "w1T": w1T,
            "adjf": adjf,
            "w2T": w2T,
            "b2r": np.ascontiguousarray(b2.reshape(1, D)),
            "ident": np.eye(P, dtype=f32).astype(bf16),
        })
    return in_maps


_NC_CACHE = None


def kernel(**inputs) -> np.ndarray:
    global _NC_CACHE, LAST_RESULT
    from concourse.bass_utils import run_bass_kernel_spmd

    in_maps = host_prep(inputs)
    if _NC_CACHE is None:
        _NC_CACHE = build_nc()
    res = run_bass_kernel_spmd(
        _NC_CACHE, in_maps, core_ids=list(range(NCORES)),
        trace=bool(os.environ.get("BASS_TRACE")),
    )
    LAST_RESULT = res
    out = np.empty((B, Q, D), np.float32)
    for c in range(NCORES):
        b, hg = c // 4, c % 4
        o = res.results[c]["out"]                       # (QS, D) token-major
        out[b, 128 * hg:128 * hg + 128, :] = o[0:128]
        out[b, 512 + 128 * hg:512 + 128 * hg + 128, :] = o[128:256]
    return out
